# revision 21
# baseline (speedup 1.0000x reference)
"""CoAttenBlock Trainium2 kernel (v2: ACT-bound pipeline).

Full inputs in, full outputs out. Shards batch (B=8) across 8 NeuronCores,
one sample per core (pure data parallel, no collectives).

Per-core math (C=64, HW=2304, 18 strips of 128 along the left position n):
  XL = WL @ [xlh;xll] + bL                      [64, 2304]
  XR = WR @ [xrh;xrl] + bR
  per strip s: aff_s = XL[:,s].T @ XR           [128, 2304] (PSUM ring)
               E_s   = exp(aff_s) -> bf16 SBUF (all 18 strips resident),
                       rowsum via activation accum_out
               r2    = 1/rowsum folded into the strip's YRT weight columns
               P12  += [YLT_s | YRT_s*r2].T @ E_s   (PSUM [128, 2304])
  colsum accumulated as csum_a (DVE, cols 0:1536) + csum_b (Pool, 1536:).
  Phase 3 runs the gate math transposed ([128, 18] layout, m = 128*q + p):
    colsumT[p,q] via per-block matmuls csum_block.T @ ones
    g1pT/g2pT via per-block matmuls P12sb_block.T @ v  (v = solve(W_r.T, gw))
    s1 = sigmoid(g1p*r1 + gb1)*r1, s2 = sigmoid(g2p + gb2)  (all [128,18])
    S12[c, 128q+p] = s12T[p, q(c)] via broadcast-lhsT matmuls vs identity
    out = W_l @ X + I @ (P12sb * S12) + b  (accumulated in PSUM, bias on ACT)

E/Wc are bf16 (keeps 18 E strips in SBUF and speeds DVE); everything on the
rowsum/colsum/P12 accumulation paths stays f32. float32r single-pass mode for
the f32 matmuls; producers of f32r matmul inputs write through f32r-typed APs.
"""

import os
import sys

import numpy as np

if os.path.isdir("/opt/trn_rl_repo") and "/opt/trn_rl_repo" not in sys.path:
    sys.path.insert(0, "/opt/trn_rl_repo")

import concourse.bass as bass
import concourse.tile as tile
from concourse import bacc, mybir
from concourse.bass_utils import run_bass_kernel_spmd

B, C, H, W = 8, 64, 48, 48
HW = H * W            # 2304
C2 = 2 * C            # 128
NSTRIP = HW // 128    # 18
F32 = mybir.dt.float32
F32R = mybir.dt.float32r
BF16 = mybir.dt.bfloat16
AF = mybir.ActivationFunctionType

# cpack column map (single staged const tensor [128, NCPACK] f32)
CP_WLORT = 0      # [0:64, 0:64]    wloRT
CP_WRORT = 64     # [0:64, 64:128]  wroRT
CP_WLOLT = 128    # [0:64, 128:192] wloLT
CP_WROLT = 192    # [0:64, 192:256] wroLT
CP_VLR = 256      # [128, 1]
CP_BLR = 257      # bL (0:64) | bR (64:128)
CP_BLRO = 258     # bLo | bRo
CP_GB1 = 259      # gateL_b replicated
CP_GB2 = 260      # gateR_b replicated
CP_ID64B = 261    # [128, 64] I64 stacked twice
CP_ID128 = 325    # [128, 128] I128
CP_BRO0 = 453     # bRo at partitions 0:64 (matmul dst must be base-0)
NCPACK = 454


def chunks(total, step=512):
    out = []
    c0 = 0
    while c0 < total:
        out.append((c0, min(step, total - c0)))
        c0 += step
    return out


CH_2304 = chunks(2304)            # 4x512 + 256
CSPLIT = 1152                     # csum_a (DVE) cols [0:1152), csum_b rest


def r(ap):
    return ap.bitcast(F32R)


def build_nc():
    nc = bacc.Bacc("TRN2", target_bir_lowering=False, debug=False)

    x2l_d = nc.dram_tensor("x2l", [C2, HW], F32, kind="ExternalInput").ap()
    x2r_d = nc.dram_tensor("x2r", [C2, HW], F32, kind="ExternalInput").ap()
    wlrT_d = nc.dram_tensor("wlrT", [C2, C2], F32, kind="ExternalInput").ap()
    cpack_d = nc.dram_tensor("cpack", [C2, NCPACK], F32,
                             kind="ExternalInput").ap()
    selpack_d = nc.dram_tensor("selpack", [2 * NSTRIP, HW], BF16,
                               kind="ExternalInput").ap()

    out_l_d = nc.dram_tensor("out_l", [C, HW], F32, kind="ExternalOutput").ap()
    out_r_d = nc.dram_tensor("out_r", [C, HW], F32, kind="ExternalOutput").ap()

    with tile.TileContext(nc) as tc:
        import contextlib

        with contextlib.ExitStack() as outer:
            consts = outer.enter_context(tc.tile_pool(name="consts", bufs=1))
            big = outer.enter_context(tc.tile_pool(name="big", bufs=1))
            epool = outer.enter_context(tc.tile_pool(name="epool", bufs=NSTRIP))
            smalls = outer.enter_context(tc.tile_pool(name="smalls", bufs=3))
            ph3sb = outer.enter_context(tc.tile_pool(name="ph3sb", bufs=2))

            # ---- big SBUF tensors ----
            x2l = big.tile([C2, HW], F32)
            x2r = big.tile([C2, HW], F32)
            XL = big.tile([C, HW], F32)
            XR = big.tile([C, HW], F32)
            Wc = big.tile([C2, HW], BF16)      # 18 strips of [YLT | YRT]
            csum_a = big.tile([C2, CSPLIT], F32)       # DVE accumulator
            csum_b = big.tile([C2, HW - CSPLIT], F32)  # Pool accumulator
            P12sb = big.tile([C2, HW], F32)    # drained P1 (0:64) / P2 (64:128)
            outLR = big.tile([C2, HW], F32)

            # ---- constants / weights ----
            wlrT = consts.tile([C2, C2], F32)
            cpack = consts.tile([C2, NCPACK], F32)
            selpack = consts.tile([2 * NSTRIP, HW], BF16)
            id128b = consts.tile([C2, C2], BF16)
            id64b = consts.tile([C2, C], BF16)
            ones128 = consts.tile([C2, 1], F32)

            # DMA order is the phase-1 critical path: the aff pipeline needs
            # ALL of XR but only strip 0 of XL, so x2r streams first on the
            # SP queue while the ACT queue brings x2l chunk 0 + consts.
            nc.sync.dma_start(out=r(wlrT), in_=r(wlrT_d))
            for j, (c0, cn) in enumerate(CH_2304):
                nc.sync.dma_start(out=r(x2r[:, c0:c0 + cn]),
                                  in_=r(x2r_d[:, c0:c0 + cn]))
            nc.scalar.dma_start(out=r(x2l[:, 0:512]), in_=r(x2l_d[:, 0:512]))
            nc.scalar.dma_start(out=r(cpack), in_=r(cpack_d))
            for c0, cn in CH_2304[1:]:
                nc.gpsimd.dma_start(out=r(x2l[:, c0:c0 + cn]),
                                    in_=r(x2l_d[:, c0:c0 + cn]))
            nc.gpsimd.dma_start(out=selpack, in_=selpack_d)
            nc.vector.memset(ones128, 1.0)
            nc.vector.tensor_copy(id128b, cpack[:, CP_ID128:CP_ID128 + C2])
            nc.vector.tensor_copy(id64b, cpack[:, CP_ID64B:CP_ID64B + C])
            wloRT = cpack[0:C, CP_WLORT:CP_WLORT + C]
            wroRT = cpack[0:C, CP_WRORT:CP_WRORT + C]
            wloLT = cpack[0:C, CP_WLOLT:CP_WLOLT + C]
            wroLT = cpack[0:C, CP_WROLT:CP_WROLT + C]
            vlr = cpack[:, CP_VLR:CP_VLR + 1]
            bL = cpack[0:C, CP_BLR:CP_BLR + 1]
            bR = cpack[C:C2, CP_BLR:CP_BLR + 1]
            bLo = cpack[0:C, CP_BLRO:CP_BLRO + 1]
            bRo0 = cpack[0:C, CP_BRO0:CP_BRO0 + 1]
            gb1 = cpack[:, CP_GB1:CP_GB1 + 1]
            gb2 = cpack[:, CP_GB2:CP_GB2 + 1]

            if True:
                affp = outer.enter_context(
                    tc.tile_pool(name="affp", bufs=1, space="PSUM"))
                p12p_ctx = contextlib.ExitStack()
                p12p = p12p_ctx.enter_context(
                    tc.tile_pool(name="p12p", bufs=1, space="PSUM"))
                P12 = p12p.tile([C2, HW], F32)  # 5 banks, lives phase 1+2
                ring = affp.tile([C2, 1536], F32, tag="ring", name="aff_ring")
                # after the last exp frees the ring, its banks host the small
                # phase-3 psum scalars (one per bank; matmul outs stay in-bank)
                colT = ring[:, 0:NSTRIP]
                g12pT = ring[:, 512:512 + 2 * NSTRIP]
                sTb = ring[0:2 * NSTRIP, 1024:1088].bitcast(BF16)  # [36,128]

                # ---- phase 1: convs + Y-strip builds ----
                # Y_t pair = [(W_r@XL_strip).T | (W_r@XR_strip).T] lands in
                # P12 scratch cols 128t:128t+128, one cast-copy to bf16 Wc.
                def emit_y(t):
                    ysl = slice(128 * t, 128 * t + 64)
                    nc.tensor.matmul(P12[:, ysl],
                                     r(XL[:, 128 * t:128 * t + 128]),
                                     r(wloRT), start=True, stop=True)
                    ysr = slice(128 * t + 64, 128 * t + 128)
                    nc.tensor.matmul(P12[:, ysr],
                                     r(XR[:, 128 * t:128 * t + 128]),
                                     r(wroRT), start=True, stop=True)
                    wsl = slice(128 * t, 128 * t + 128)
                    nc.vector.tensor_copy(Wc[:, wsl], P12[:, wsl])

                # XR convs first (aff needs all of XR); XL chunk 0 next so
                # strip 0 can start; XL chunks 1-4 + Y builds trail behind.
                for j, (c0, cn) in enumerate(CH_2304):
                    rsl = (j % 3) * 512
                    nc.tensor.matmul(ring[0:C, rsl:rsl + cn], r(wlrT[:, C:C2]),
                                     r(x2r[:, c0:c0 + cn]), start=True,
                                     stop=True)
                    nc.vector.tensor_scalar_add(r(XR[:, c0:c0 + cn]),
                                                ring[0:C, rsl:rsl + cn], bR)
                for j, (c0, cn) in enumerate(CH_2304):
                    nc.tensor.matmul(P12[0:C, c0:c0 + cn], r(wlrT[:, 0:C]),
                                     r(x2l[:, c0:c0 + cn]), start=True,
                                     stop=True)
                    if j == 0:
                        nc.scalar.activation(r(XL[:, c0:c0 + cn]),
                                             P12[0:C, c0:c0 + cn],
                                             AF.Identity, bias=bL, scale=1.0)
                    else:
                        nc.vector.tensor_scalar_add(r(XL[:, c0:c0 + cn]),
                                                    P12[0:C, c0:c0 + cn], bL)
                    for t in range(4 * j, min(4 * j + 4, NSTRIP)):
                        emit_y(t)

                # ---- phase 2: strip loop, 768-wide exp groups ----
                # ring = two 768-col regions (A=[0:768), B=[768:1536)) used
                # alternately by consecutive groups (3 per strip), so the affs
                # for group G run while group G-1 is still in the ACT engine.
                # bacc_{s-2} pieces are PE filler between aff groups.
                r2s = {}
                Es = {}

                def bacc_piece(sb, c0, cn):
                    nc.tensor.matmul(P12[:, c0:c0 + cn],
                                     Wc[:, 128 * sb:128 * sb + 128],
                                     Es[sb][:, c0:c0 + cn],
                                     start=(sb == 0), stop=(sb == NSTRIP - 1))

                gctr = [0]

                def emit_group(s, g, E, rs):
                    m0 = 768 * g
                    lhs_aff = r(XL[:, 128 * s:128 * s + 128])
                    if gctr[0] % 2 == 0:   # region A: 512@0, 256@512
                        pieces = [(m0, 512, 0), (m0 + 512, 256, 512)]
                        r0 = 0
                    else:                  # region B: 256@768, 512@1024
                        pieces = [(m0, 256, 768), (m0 + 256, 512, 1024)]
                        r0 = 768
                    for p0, pn, rof in pieces:
                        nc.tensor.matmul(ring[:, rof:rof + pn],
                                         lhs_aff, r(XR[:, p0:p0 + pn]),
                                         start=True, stop=True)
                    nc.scalar.activation(E[:, m0:m0 + 768],
                                         ring[:, r0:r0 + 768], AF.Exp,
                                         accum_out=rs[:, g:g + 1])
                    gctr[0] += 1

                def emit_strip(s):
                    E = epool.tile([C2, HW], BF16, tag="e", name=f"E_{s}")
                    rs = smalls.tile([C2, 3], F32, tag="rs", name=f"rs_{s}")
                    sb = s - 2
                    for g in range(3):
                        emit_group(s, g, E, rs)
                        if sb >= 0:
                            for c0, cn in (CH_2304[2 * g:2 * g + 2]
                                           if g < 2 else CH_2304[4:]):
                                bacc_piece(sb, c0, cn)
                    rowsum = smalls.tile([C2, 1], F32, tag="rowsum",
                                         name=f"rowsum_{s}")
                    r2 = smalls.tile([C2, 1], F32, tag="r2", name=f"r2_{s}",
                                     bufs=4)
                    nc.vector.tensor_reduce(rowsum, rs,
                                            axis=mybir.AxisListType.X,
                                            op=mybir.AluOpType.add)
                    nc.vector.reciprocal(r2, rowsum)
                    r2s[s] = r2
                    wright = Wc[:, 128 * s + 64:128 * s + 128]
                    nc.vector.tensor_scalar_mul(wright, wright, r2)
                    # colsum accumulate: DVE takes [0:1152), Pool the rest
                    if s == 0:
                        nc.vector.tensor_copy(csum_a, E[:, 0:CSPLIT])
                        nc.gpsimd.tensor_copy(csum_b, E[:, CSPLIT:HW])
                    else:
                        nc.vector.tensor_add(csum_a, csum_a, E[:, 0:CSPLIT])
                        nc.gpsimd.tensor_add(csum_b, csum_b, E[:, CSPLIT:HW])
                    return E

                for s in range(NSTRIP):
                    Es[s] = emit_strip(s)
                for c0, cn in CH_2304:
                    bacc_piece(NSTRIP - 2, c0, cn)
                # colsum-transpose matmuls only need csum: run during the
                # final bacc so the phase-3 front shortens
                for q in range(NSTRIP):
                    if 128 * q + 128 <= CSPLIT:
                        src = csum_a[:, 128 * q:128 * q + 128]
                    else:
                        src = csum_b[:, 128 * q - CSPLIT:128 * q - CSPLIT + 128]
                    nc.tensor.matmul(colT[:, q:q + 1], src, ones128,
                                     start=True, stop=True)
                for c0, cn in CH_2304:
                    bacc_piece(NSTRIP - 1, c0, cn)

                # drain P12 to SBUF (GPSIMD can't read PSUM: DVE/ACT halves)
                nc.vector.tensor_copy(P12sb[:, 0:1152], P12[:, 0:1152])
                nc.scalar.copy(P12sb[:, 1152:2304], P12[:, 1152:2304])
                p12p_ctx.close()

            # ---- phase 3 ----
            with tc.tile_pool(name="ph3r", bufs=2, space="PSUM") as ph3r:
                OLs, ORs = {}, {}

                def emit_conv(ci):
                    c0, cn = CH_2304[ci]
                    OL = ph3r.tile([C, cn], F32, tag="OL", name=f"OL_{ci}",
                                   padded_shape=[C, 512])
                    nc.tensor.matmul(OL, r(wloLT), r(XL[:, c0:c0 + cn]),
                                     start=True, stop=False)
                    OR_ = ph3r.tile([C, cn], F32, tag="OR", name=f"OR_{ci}",
                                    padded_shape=[C, 512])
                    nc.tensor.matmul(OR_, r(wroLT), r(XR[:, c0:c0 + cn]),
                                     start=True, stop=False)
                    OLs[ci], ORs[ci] = OL, OR_

                # conv halves of the first chunks run while P12 drains
                emit_conv(0)
                emit_conv(1)
                r1T = ph3sb.tile([C2, NSTRIP], F32, name="r1T")
                nc.vector.reciprocal(r1T, colT)
                for q in range(NSTRIP):
                    blk = slice(128 * q, 128 * q + 128)
                    nc.tensor.matmul(g12pT[:, q:q + 1], P12sb[0:C, blk],
                                     vlr[0:C], start=True, stop=True)
                    nc.tensor.matmul(g12pT[:, NSTRIP + q:NSTRIP + q + 1],
                                     P12sb[C:C2, blk], vlr[C:C2],
                                     start=True, stop=True)
                g1preT = ph3sb.tile([C2, NSTRIP], F32, name="g1preT")
                nc.vector.tensor_mul(g1preT, g12pT[:, 0:NSTRIP], r1T)
                g1T = ph3sb.tile([C2, NSTRIP], F32, name="g1T")
                nc.scalar.activation(g1T, g1preT, AF.Sigmoid, bias=gb1,
                                     scale=1.0)
                s12T = ph3sb.tile([C2, 2 * NSTRIP], BF16, name="s12T")
                nc.vector.tensor_mul(s12T[:, 0:NSTRIP], g1T, r1T)
                nc.scalar.activation(s12T[:, NSTRIP:2 * NSTRIP],
                                     g12pT[:, NSTRIP:2 * NSTRIP], AF.Sigmoid,
                                     bias=gb2, scale=1.0)
                # transpose to [36, 128] rows (q-major) so S12 can be built by
                # per-block selector matmuls against SBUF data
                nc.tensor.transpose(sTb, s12T, id128b)
                sT_sb = ph3sb.tile([2 * NSTRIP, C2], BF16, name="sT_sb")
                nc.vector.tensor_copy(sT_sb, sTb)

                for ci, (c0, cn) in enumerate(CH_2304):
                    if ci not in OLs:
                        emit_conv(ci)
                    S12 = ph3r.tile([C2, cn], F32, tag="S12", name=f"S12_{ci}",
                                    bufs=1, padded_shape=[C2, 512])
                    for b in range(cn // 128):
                        m0 = c0 + 128 * b
                        nc.tensor.matmul(S12[:, 128 * b:128 * b + 128],
                                         selpack[:, m0:m0 + 128], sT_sb,
                                         start=True, stop=True)
                    t12 = ph3sb.tile([C2, cn], BF16, tag="t12",
                                     name=f"t12_{ci}", padded_shape=[C2, 512])
                    nc.vector.tensor_mul(t12, P12sb[:, c0:c0 + cn], S12)
                    nc.tensor.matmul(OLs[ci], id64b[0:C], t12[0:C, :],
                                     start=False, stop=True)
                    nc.scalar.activation(outLR[0:C, c0:c0 + cn], OLs[ci],
                                         AF.Identity, bias=bLo, scale=1.0)
                    nc.tensor.matmul(ORs[ci], id64b[C:C2], t12[C:C2, :],
                                     start=False, stop=True)
                    nc.vector.tensor_scalar_add(outLR[C:C2, c0:c0 + cn],
                                                ORs[ci], bRo0)
                    nc.sync.dma_start(out=out_l_d[:, c0:c0 + cn],
                                      in_=outLR[0:C, c0:c0 + cn])
                    nc.gpsimd.dma_start(out=out_r_d[:, c0:c0 + cn],
                                        in_=outLR[C:C2, c0:c0 + cn])

    nc.compile()
    return nc


_NC_CACHE = {}


def _get_nc():
    if "nc" not in _NC_CACHE:
        _NC_CACHE["nc"] = build_nc()
    return _NC_CACHE["nc"]


def _prep_shared(concaL_w, concaL_b, concaR_w, concaR_b,
                 gateL_w, gateL_b, gateR_w, gateR_b,
                 concaLo_w, concaLo_b, concaRo_w, concaRo_b):
    f = np.float32
    wloR = np.asarray(concaLo_w)[:, C:].astype(np.float64)
    wroR = np.asarray(concaRo_w)[:, C:].astype(np.float64)
    vL = np.linalg.solve(wloR.T, np.asarray(gateL_w).astype(np.float64).reshape(C))
    vR = np.linalg.solve(wroR.T, np.asarray(gateR_w).astype(np.float64).reshape(C))
    wlrT = np.concatenate([np.asarray(concaL_w).T, np.asarray(concaR_w).T],
                          axis=1)

    cpack = np.zeros((C2, NCPACK), dtype=f)
    cpack[0:C, CP_WLORT:CP_WLORT + C] = wloR.T
    cpack[0:C, CP_WRORT:CP_WRORT + C] = wroR.T
    cpack[0:C, CP_WLOLT:CP_WLOLT + C] = np.asarray(concaLo_w)[:, :C].T
    cpack[0:C, CP_WROLT:CP_WROLT + C] = np.asarray(concaRo_w)[:, :C].T
    cpack[:, CP_VLR] = np.concatenate([vL, vR])
    cpack[0:C, CP_BLR] = np.asarray(concaL_b).reshape(C)
    cpack[C:C2, CP_BLR] = np.asarray(concaR_b).reshape(C)
    cpack[0:C, CP_BLRO] = np.asarray(concaLo_b).reshape(C)
    cpack[C:C2, CP_BLRO] = np.asarray(concaRo_b).reshape(C)
    cpack[0:C, CP_BRO0] = np.asarray(concaRo_b).reshape(C)
    cpack[:, CP_GB1] = np.asarray(gateL_b).reshape(())
    cpack[:, CP_GB2] = np.asarray(gateR_b).reshape(())
    eye = np.eye(C, dtype=f)
    cpack[0:C, CP_ID64B:CP_ID64B + C] = eye
    cpack[C:C2, CP_ID64B:CP_ID64B + C] = eye
    cpack[:, CP_ID128:CP_ID128 + C2] = np.eye(C2, dtype=f)
    import ml_dtypes
    # selpack[k, 128q+c] = 1 iff (c<64 and k==q) or (c>=64 and k==18+q):
    # S12 block q = selpack[:, blk].T @ sT_sb broadcasts s1/s2 rows of sT
    # over the channel halves of the output.
    selpack = np.zeros((2 * NSTRIP, HW), dtype=np.float32)
    for q in range(NSTRIP):
        selpack[q, 128 * q:128 * q + 64] = 1.0
        selpack[NSTRIP + q, 128 * q + 64:128 * q + 128] = 1.0
    return {
        "wlrT": np.ascontiguousarray(wlrT, dtype=f),
        "cpack": np.ascontiguousarray(cpack, dtype=f),
        "selpack": np.ascontiguousarray(selpack.astype(ml_dtypes.bfloat16)),
    }


def kernel(xlh, xll, xrh, xrl,
           concaL_w, concaL_b, concaR_w, concaR_b,
           gateL_w, gateL_b, gateR_w, gateR_b,
           concaLo_w, concaLo_b, concaRo_w, concaRo_b,
           _return_results=False):
    nc = _get_nc()
    shared = _prep_shared(concaL_w, concaL_b, concaR_w, concaR_b,
                          gateL_w, gateL_b, gateR_w, gateR_b,
                          concaLo_w, concaLo_b, concaRo_w, concaRo_b)
    xlh = np.asarray(xlh, dtype=np.float32)
    xll = np.asarray(xll, dtype=np.float32)
    xrh = np.asarray(xrh, dtype=np.float32)
    xrl = np.asarray(xrl, dtype=np.float32)

    in_maps = []
    for c in range(B):
        x2l = np.concatenate([xlh[c].reshape(C, HW), xll[c].reshape(C, HW)], axis=0)
        x2r = np.concatenate([xrh[c].reshape(C, HW), xrl[c].reshape(C, HW)], axis=0)
        m = dict(shared)
        m["x2l"] = np.ascontiguousarray(x2l)
        m["x2r"] = np.ascontiguousarray(x2r)
        in_maps.append(m)

    # The first execution of a freshly compiled NEFF occasionally hits a
    # transient NRT_EXEC_UNIT_UNRECOVERABLE on this axon setup; an immediate
    # re-dispatch of the same executable has always succeeded, so retry.
    res = None
    for attempt in range(3):
        try:
            res = run_bass_kernel_spmd(nc, in_maps, list(range(B)))
            break
        except Exception:
            if attempt == 2:
                raise
            import time as _time
            _time.sleep(2.0)
    out_L = np.stack([res.results[c]["out_l"].reshape(C, H, W) for c in range(B)])
    out_R = np.stack([res.results[c]["out_r"].reshape(C, H, W) for c in range(B)])
    if _return_results:
        return (out_L, out_R), res
    return (out_L, out_R)
